# revision 1
# baseline (speedup 1.0000x reference)
"""AttnBlock (GroupNorm -> single-head self-attention -> residual) on 8 TRN2 cores.

Sharding: B=4 batch elements x 2 query-token halves = 8 cores (SPMD, no
collectives).  Each core receives the full (rolled) channel-major batch
element x^T [C=256, HW=4096], computes GroupNorm + k/v for all 4096
tokens, and q/scores/attention/out-proj for its 2048-token half.  Odd
cores get x rolled by -2048 tokens; attention is permutation-invariant
over keys, so their first 2048 tokens are the original tokens 2048:4096.

Layout is channel-major throughout (tokens on the free axis), which makes
every matmul transpose-free:
  hs^T = GN(x^T)                          [C, N]
  q^T = Wq^T.T @ hs^T  (lhsT=Wq^T)        [C, NQ]
  k^T likewise                            [C, N]
  v   = hs^T.T @ Wv^T  (lhsT=hs^T)        [N, C]   (row-major)
  S^T = k^T.T @ q^T    (lhsT=k^T)         [N, NQ]  (keys on partitions)
  P^T = exp(S^T/16)    (bf16)             softmax numerator, no max-sub
  Z   = ones.T @ P^T   (M=1 matmul)       [1, NQ]  denominators
  o^T = v.T @ P^T      (lhsT=v)           [C, NQ]; scaled by 1/Z
  out^T = Wo^T.T @ o^T + bo               [C, NQ]
  final = (x^T + out^T) / sqrt(2)
"""

import numpy as np
import ml_dtypes

import concourse.bass as bass
import concourse.tile as tile
from concourse import bacc, mybir
from concourse.bass_utils import run_bass_kernel_spmd

dt = mybir.dt
F32, F32R, BF16 = dt.float32, dt.float32r, dt.bfloat16
AF = mybir.ActivationFunctionType
ALU = mybir.AluOpType

P = 128          # partitions
C = 256          # channels
N = 4096         # tokens per batch element (64*64)
NQ = 2048        # query tokens per core
NSTRIP = 256     # query-token strip width
NS = NQ // NSTRIP  # 8 strips
MT = N // P      # 32 key m-tiles
GS = 8           # channels per group (256 / 32 groups)
EPS = 1e-6
ISCALE = 1.0 / 16.0      # attention scale c**-0.5
RS2 = float(2.0 ** -0.5)  # output residual scale

_prog_cache = {}


def _build_nc():
    nc = bacc.Bacc("TRN2", target_bir_lowering=False, debug=False, num_devices=8)

    def inp(name, shape, d=F32):
        return nc.dram_tensor(name, shape, d, kind="ExternalInput").ap()

    xt_d = inp("xt", [2, P, N])            # [c_half, c_in, n]
    wq_d = inp("wqT", [2, P, C])           # [ci_half, ci_in, c_out] = Wq.T
    wk_d = inp("wkT", [2, P, C])
    wv_d = inp("wvT", [2, P, C])
    wo_d = inp("woT", [2, P, C])
    bq_d = inp("bqp", [P, 2])              # [c_out_in, c_out_half]
    bk_d = inp("bkp", [P, 2])
    bos_d = inp("bosp", [P, 2])            # bo * 2^-0.5, packed
    bv_d = inp("bv", [1, C])
    gnw_d = inp("gnw", [P, 2])
    gnb_d = inp("gnb", [P, 2])
    amat_d = inp("amat", [P, P])           # block-diag 8x8 of 1/8
    ones1_d = inp("ones1", [1, P])
    onesm_d = inp("onesm", [P, 1], BF16)
    sel4_d = inp("sel4", [P, 1])           # 1.0 at partitions 0/32/64/96
    out_d = nc.dram_tensor("out", [2, P, NQ], F32, kind="ExternalOutput").ap()

    with tile.TileContext(nc) as tc:
        with (
            tc.tile_pool(name="singles", bufs=1) as singles,
            tc.tile_pool(name="xpool", bufs=1) as xpool,
            tc.tile_pool(name="hsfin", bufs=1) as hsfin,
            tc.tile_pool(name="qk", bufs=1) as qk,
            tc.tile_pool(name="vpool", bufs=1) as vpool,
            tc.tile_pool(name="espool", bufs=2) as espool,
            tc.tile_pool(name="opool", bufs=1) as opool,
            tc.tile_pool(name="small", bufs=2) as small,
            tc.tile_pool(name="zf", bufs=2) as zfpool,
            tc.tile_pool(name="ps", bufs=2, space="PSUM") as ps,
            tc.tile_pool(name="po", bufs=2, space="PSUM") as po,
            tc.tile_pool(name="pm", bufs=1, space="PSUM") as pm,
            tc.tile_pool(name="pz", bufs=1, space="PSUM") as pz,
        ):
            # ---- x load first: keep the HWDGE queues free of weight
            # traffic so GroupNorm stats start as soon as chunks land ----
# ---- x load (chunked; bn_stats pipelined behind each chunk) ----
            xt0 = xpool.tile([P, N], F32, tag="xt0")
            xt1 = xpool.tile([P, N], F32, tag="xt1")
            xts = (xt0, xt1)
            _dmae = [nc.sync, nc.scalar]
            for t in range(2):
                for h in range(4):
                    _dmae[h % 2].dma_start(
                        xts[t][:, h * 1024:(h + 1) * 1024],
                        xt_d[t, :, h * 1024:(h + 1) * 1024])

            # ---- constants / weights ----
            wq = singles.tile([P, 2, C], F32R)
            for _ko in range(2):
                nc.gpsimd.dma_start(wq[:, _ko, :], wq_d[_ko].bitcast(F32R))
            wk = singles.tile([P, 2, C], F32R)
            for _ko in range(2):
                nc.gpsimd.dma_start(wk[:, _ko, :], wk_d[_ko].bitcast(F32R))
            wv = singles.tile([P, 2, C], F32R)
            for _ko in range(2):
                nc.gpsimd.dma_start(wv[:, _ko, :], wv_d[_ko].bitcast(F32R))
            wo = singles.tile([P, 2, C], F32R)
            for _ko in range(2):
                nc.gpsimd.dma_start(wo[:, _ko, :], wo_d[_ko].bitcast(F32R))
            bq = singles.tile([P, 2], F32)
            nc.gpsimd.dma_start(bq[:], bq_d)
            bk = singles.tile([P, 2], F32)
            nc.gpsimd.dma_start(bk[:], bk_d)
            bos = singles.tile([P, 2], F32)
            nc.gpsimd.dma_start(bos[:], bos_d)
            gnw = singles.tile([P, 2], F32)
            nc.gpsimd.dma_start(gnw[:], gnw_d)
            gnb = singles.tile([P, 2], F32)
            nc.gpsimd.dma_start(gnb[:], gnb_d)
            amat = singles.tile([P, P], F32R)
            nc.gpsimd.dma_start(amat[:], amat_d.bitcast(F32R))
            ones1 = singles.tile([1, P], F32R)
            nc.gpsimd.dma_start(ones1[:], ones1_d.bitcast(F32R))
            onesm = singles.tile([P, 1], BF16)
            nc.gpsimd.dma_start(onesm[:], onesm_d)
            sel4 = singles.tile([P, 1], F32R)
            nc.gpsimd.dma_start(sel4[:], sel4_d.bitcast(F32R))
            # bv broadcast to all partitions (stride-0 partition DMA)
            bvrep = singles.tile([P, C], F32)
            bv_b = bass.AP(tensor=bv_d.tensor, offset=bv_d.offset,
                           ap=[[0, P], bv_d.ap[1]])
            nc.gpsimd.dma_start(out=bvrep[:], in_=bv_b)
            epsap = singles.tile([P, 1], F32)
            nc.vector.memset(epsap[:], EPS)

            # ---- GroupNorm (channel-major; stats per channel then 8-chan groups) ----
            hs = hsfin.tile([P, 2, N], F32R, tag="hsfin")
            for t in range(2):
                st = small.tile([P, 8, 6], F32, tag="gnst")
                xre = xts[t][:, :].rearrange("p (s f) -> p s f", f=512)
                for sg in range(8):
                    nc.vector.bn_stats(st[:, sg, :], xre[:, sg, :])
                mv = small.tile([P, 2], F32, tag="gnmv")
                nc.vector.bn_aggr(mv[:], st[:])
                # stats2 = [mu, E[x^2]] per channel, rounded to f32r for the matmul
                musq = small.tile([P, 1], F32, tag="gnmusq")
                nc.vector.tensor_mul(musq[:], mv[:, 0:1], mv[:, 0:1])
                stats2 = small.tile([P, 2], F32R, tag="gnst2")
                nc.vector.tensor_copy(stats2[:, 0:1], mv[:, 0:1])
                nc.vector.tensor_add(stats2[:, 1:2], mv[:, 1:2], musq[:])
                # group-aggregate (mean over 8 channels) and broadcast back
                gp = pm.tile([P, 512], F32, tag="pm")
                nc.tensor.matmul(gp[:, 0:2], amat[:], stats2[:], start=True, stop=True)
                gs = small.tile([P, 2], F32, tag="gnagg")
                nc.vector.tensor_copy(gs[:], gp[:, 0:2])
                gvar = small.tile([P, 1], F32, tag="gnvar")
                gmusq = small.tile([P, 1], F32, tag="gnmusq2")
                nc.vector.tensor_mul(gmusq[:], gs[:, 0:1], gs[:, 0:1])
                nc.vector.tensor_tensor(gvar[:], gs[:, 1:2], gmusq[:], ALU.subtract)
                # rstd = exp(-0.5 * ln(var + eps))  (same ACT table set as softmax exp)
                lnv = small.tile([P, 1], F32, tag="gnln")
                nc.scalar.activation(lnv[:], gvar[:], AF.Ln, bias=epsap[:], scale=1.0)
                rstd = small.tile([P, 1], F32, tag="gnrstd")
                nc.scalar.activation(rstd[:], lnv[:], AF.Exp, bias=0.0, scale=-0.5)
                alpha = small.tile([P, 1], F32, tag="gnalpha")
                nc.vector.tensor_mul(alpha[:], rstd[:], gnw[:, t:t + 1])
                atmp = small.tile([P, 1], F32, tag="gnatmp")
                nc.vector.tensor_mul(atmp[:], gs[:, 0:1], alpha[:])
                beta = small.tile([P, 1], F32, tag="gnbeta")
                nc.vector.tensor_tensor(beta[:], gnb[:, t:t + 1], atmp[:], ALU.subtract)
                for hh in range(2):
                    nc.vector.tensor_scalar(hs[:, t, hh * 2048:(hh + 1) * 2048],
                                            xts[t][:, hh * 2048:(hh + 1) * 2048],
                                            alpha[:], beta[:], ALU.mult, ALU.add)

            # ---- projections ----
            qT = qk.tile([P, 2, NQ], F32R, tag="qT")
            kT = qk.tile([P, 2, N], F32R, tag="kT")
            for (wt, bt, dst, nblk) in ((wq, bq, qT, NQ // 256), (wk, bk, kT, N // 256)):
                for ch in range(2):
                    for j in range(nblk // 2):
                        sp = ps.tile([P, 4, NSTRIP], F32, tag="ps")
                        for i in range(2):
                            b = 2 * j + i
                            for ko in range(2):
                                nc.tensor.matmul(
                                    sp[:, i, :],
                                    wt[:, ko, ch * P:(ch + 1) * P],
                                    hs[:, ko, b * 256:(b + 1) * 256],
                                    start=(ko == 0), stop=(ko == 1))
                        nc.vector.tensor_scalar(
                            dst[:, ch, 2 * j * 256:(2 * j + 2) * 256],
                            sp[:, 0:2, :].rearrange("p a b -> p (a b)"),
                            bt[:, ch:ch + 1], None, ALU.add)
            v = vpool.tile([P, MT, C], BF16)
            for m in range(MT):
                if m % 2 == 0:
                    vpt = pm.tile([P, 512], F32, tag="pm", name=f"vp{m}")
                    vp = vpt[:, 0:C]
                else:
                    vpt = po.tile([P, 2, NSTRIP], F32, tag="po", name=f"vp{m}")
                    vp = vpt[:, 0, :]
                for ko in range(2):
                    nc.tensor.matmul(vp, hs[:, ko, m * P:(m + 1) * P],
                                     wv[:, ko, :], start=(ko == 0), stop=(ko == 1))
                nc.vector.tensor_add(v[:, m, :], vp, bvrep[:])

            # ---- attention strips ----
            final = hsfin.tile([P, 2, NQ], F32, tag="hsfin")
            for s in range(NS):
                ns = slice(s * NSTRIP, (s + 1) * NSTRIP)
                es = espool.tile([P, MT, NSTRIP], BF16, tag="es")
                for j in range(MT // 4):
                    sp = ps.tile([P, 4, NSTRIP], F32, tag="ps")
                    for i in range(4):
                        m = 4 * j + i
                        for ko in range(2):
                            nc.tensor.matmul(sp[:, i, :], kT[:, ko, m * P:(m + 1) * P],
                                             qT[:, ko, ns], start=(ko == 0), stop=(ko == 1))
                    nc.scalar.activation(es[:, 4 * j:4 * j + 4, :], sp[:],
                                         AF.Exp, bias=0.0, scale=ISCALE)
                # softmax denominators: Z = ones.T @ P^T, 4 col-packed M=1 chains
                zp = pz.tile([P, NSTRIP], F32, tag="pz")
                for j in range(MT // 4):
                    for c in range(4):
                        nc.tensor.matmul(zp[32 * c:32 * c + 1, :], onesm[:],
                                         es[:, 4 * j + c, :],
                                         start=(j == 0), stop=(j == MT // 4 - 1),
                                         tile_position=(0, 32 * c))
                # attn @ v
                op = po.tile([P, 2, NSTRIP], F32, tag="po")
                for ch in range(2):
                    for m in range(MT):
                        nc.tensor.matmul(op[:, ch, :], v[:, m, ch * P:(ch + 1) * P],
                                         es[:, m, :], start=(m == 0), stop=(m == MT - 1))
                # Z = sel4.T @ zsb picks+sums the 4 packed rows (others hold garbage)
                zsb = small.tile([P, NSTRIP], F32R, tag="zsb")
                nc.vector.tensor_copy(zsb[:], zp[:])
                zqt = pm.tile([P, 512], F32, tag="pm", name=f"zq{s}")
                nc.tensor.matmul(zqt[0:1, 0:NSTRIP], sel4[:], zsb[:],
                                 start=True, stop=True)
                # 1/Z on DVE (avoids ACT table-set thrash), broadcast via K=1 matmul
                rz = small.tile([1, NSTRIP], F32R, tag="rz")
                with nc.allow_low_precision(reason="f32r rounding of 1/Z"):
                    nc.vector.reciprocal(rz[:], zqt[0:1, 0:NSTRIP])
                rp = pm.tile([P, 512], F32, tag="pm")
                nc.tensor.matmul(rp[:, 0:NSTRIP], ones1[:], rz[:], start=True, stop=True)
                rzs = small.tile([P, NSTRIP], F32, tag="rzs")
                nc.vector.tensor_copy(rzs[:], rp[:, 0:NSTRIP])
                o = opool.tile([P, 2, NQ], F32R, tag="o")
                for ch in range(2):
                    nc.vector.tensor_mul(o[:, ch, ns], op[:, ch, :], rzs[:])
                # out projection + bias + residual + 2^-0.5 (psum from po — free
                # here, and keeps pm's single slot off the strip critical path)
                op2 = po.tile([P, 2, NSTRIP], F32, tag="po", name=f"op2_{s}")
                for ch in range(2):
                    for ko in range(2):
                        nc.tensor.matmul(op2[:, ch, :],
                                         wo[:, ko, ch * P:(ch + 1) * P],
                                         o[:, ko, ns], start=(ko == 0), stop=(ko == 1))
                z2 = zfpool.tile([P, 2, NSTRIP], F32, tag="zf")
                for ch in range(2):
                    nc.scalar.activation(z2[:, ch, :], op2[:, ch, :],
                                         AF.Identity, bias=bos[:, ch:ch + 1], scale=RS2)
                for t in range(2):
                    nc.vector.scalar_tensor_tensor(
                        out=final[:, t, ns], in0=xts[t][:, ns], scalar=RS2,
                        in1=z2[:, t, :], op0=ALU.mult, op1=ALU.add)
                    nc.sync.dma_start(out_d[t, :, ns], final[:, t, ns])

    nc.finalize()
    return nc


def _get_nc():
    if "nc" not in _prog_cache:
        _prog_cache["nc"] = _build_nc()
    return _prog_cache["nc"]


def _make_in_maps(x, gn_weight, gn_bias, Wq, bq, Wk, bk, Wv, bv, Wo, bo):
    x = np.asarray(x, dtype=np.float32)
    f32 = lambda a: np.ascontiguousarray(np.asarray(a, dtype=np.float32))

    def packT(b_vec):  # [256] -> [128, 2] (c_out_in, c_out_half)
        return np.ascontiguousarray(f32(b_vec).reshape(2, P).T)

    amat = np.zeros((P, P), np.float32)
    for g in range(P // GS):
        amat[g * GS:(g + 1) * GS, g * GS:(g + 1) * GS] = 1.0 / GS
    sel4 = np.zeros((P, 1), np.float32)
    sel4[[0, 32, 64, 96], 0] = 1.0

    common = {
        "wqT": f32(np.asarray(Wq).T).reshape(2, P, C),
        "wkT": f32(np.asarray(Wk).T).reshape(2, P, C),
        "wvT": f32(np.asarray(Wv).T).reshape(2, P, C),
        "woT": f32(np.asarray(Wo).T).reshape(2, P, C),
        "bqp": packT(bq),
        "bkp": packT(bk),
        "bosp": packT(np.asarray(bo, dtype=np.float32) * RS2),
        "bv": f32(bv).reshape(1, C),
        "gnw": packT(gn_weight),
        "gnb": packT(gn_bias),
        "amat": amat,
        "ones1": np.ones((1, P), np.float32),
        "onesm": np.ones((P, 1), ml_dtypes.bfloat16),
        "sel4": sel4,
    }

    in_maps = []
    for core in range(8):
        b, half = core // 2, core % 2
        xt = x[b].reshape(C, N)
        if half:
            xt = np.roll(xt, -NQ, axis=1)
        in_maps.append({"xt": np.ascontiguousarray(xt).reshape(2, P, N), **common})
    return in_maps


def _assemble(results, B):
    out = np.empty((B, C, N), np.float32)
    for core in range(2 * B):
        b, half = core // 2, core % 2
        out[b, :, half * NQ:(half + 1) * NQ] = results[core]["out"].reshape(C, NQ)
    return out.reshape(B, C, 64, 64)


def kernel(x, gn_weight, gn_bias, Wq, bq, Wk, bk, Wv, bv, Wo, bo):
    x = np.asarray(x, dtype=np.float32)
    in_maps = _make_in_maps(x, gn_weight, gn_bias, Wq, bq, Wk, bk, Wv, bv, Wo, bo)
    nc = _get_nc()
    res = run_bass_kernel_spmd(nc, in_maps, list(range(8)))
    return _assemble(res.results, x.shape[0])



# revision 6
# speedup vs baseline: 1.9448x; 1.9448x over previous
"""AttnBlock (GroupNorm -> single-head self-attention -> residual) on 8 TRN2 cores.

Sharding: B=4 batch elements x 2 query-token halves = 8 cores (SPMD, no
collectives).  Each core receives the full (rolled) channel-major batch
element x^T [C=256, HW=4096] in bf16, computes GroupNorm + k/v for all
4096 tokens, and q/scores/attention/out-proj for its 2048-token half.
Odd cores get x rolled by -2048 tokens; attention is permutation-
invariant over keys, so their first 2048 tokens are tokens 2048:4096.

The two big attention matmuls (scores and attn@v) and the softmax-
denominator chain run in fp8-e4m3 with MatmulPerfMode.DoubleRow
(K=256 packed 2-rows-per-PE-cell, 0.5 cycles/row).  Softmax numerators
use exp(s/16 - 2) so es <= ~57 < 240 (TRN fp8e4 max); the constant
offset cancels in the softmax ratio.  Projections are bf16.  Layout is
channel-major (tokens on the free axis), all matmuls transpose-free:

  hs^T = GN(x^T)  bf16                    [C, N]
  q^T = Wq^T.T @ hs^T -> fp8 (ACT cast)   [C, NQ]   (dim1 = ko ktile)
  k^T likewise -> fp8                     [C, N]
  v   = hs^T.T @ Wv^T + bv -> fp8         [N, C]    (row-major)
  S^T = DR(k^T, q^T)                      [N, NQ]   one matmul per m-tile
  es  = exp(S^T/16 - 2) -> fp8 (ACT)
  Z   = DR(ones, es) chain                [16, NQ]  (row 0 used)
  o^T = DR(v, es) chain                   [C, NQ]
  out^T = (Wo^T*2^-0.5).T @ bf16(o^T)     [C, NQ]
  final = xr + out^T * (1/Z),  xr = (x + bo) * 2^-0.5  (host-side)
"""

import numpy as np
import ml_dtypes

import concourse.bass as bass
import concourse.tile as tile
from concourse import bacc, mybir
from concourse.bass_utils import run_bass_kernel_spmd

dt = mybir.dt
F32, F32R, BF16, FP8 = dt.float32, dt.float32r, dt.bfloat16, dt.float8e4
AF = mybir.ActivationFunctionType
ALU = mybir.AluOpType
DR = mybir.MatmulPerfMode.DoubleRow

P = 128          # partitions
C = 256          # channels
N = 4096         # tokens per batch element (64*64)
NQ = 2048        # query tokens per core
NSTRIP = 256     # query-token strip width
NS = NQ // NSTRIP  # 8 strips
MT = N // P      # 32 key m-tiles
GS = 8           # channels per group (256 / 32 groups)
EPS = 1e-6
ISCALE = 1.0 / 16.0       # attention scale c**-0.5
EOFF = 2.0                # exp offset: es = exp(s/16 - EOFF), cancels in softmax
RS2 = float(2.0 ** -0.5)  # output residual scale

_prog_cache = {}


def _build_nc():
    nc = bacc.Bacc("TRN2", target_bir_lowering=False, debug=False, num_devices=8)

    def inp(name, shape, d=F32):
        return nc.dram_tensor(name, shape, d, kind="ExternalInput").ap()

    xtb_d = inp("xtb", [2, P, N], BF16)    # [c_half, c_in, n] bf16
    xr_d = inp("xr", [2, P, N])            # (x + bo) * 2^-0.5
    wq_d = inp("wqT", [2, P, C], BF16)     # [ci_half, ci_in, c_out] = Wq.T
    wk_d = inp("wkT", [2, P, C], BF16)
    wv_d = inp("wvT", [2, P, C], BF16)
    wo_d = inp("woT", [2, P, C], BF16)     # Wo.T * 2^-0.5
    bq_d = inp("bqp", [P, 2])              # [c_out_in, c_out_half]
    bk_d = inp("bkp", [P, 2])
    bv_d = inp("bv4", [1, 4 * C])          # bv tiled 4x for [P,4,C] broadcast
    gnw_d = inp("gnw", [P, 2])
    gnb_d = inp("gnb", [P, 2])
    amat_d = inp("amat", [P, P])           # block-diag 8x8 of 1/8
    ones1_d = inp("ones1", [1, P])
    out_d = nc.dram_tensor("out", [2, P, NQ], F32, kind="ExternalOutput").ap()

    with tile.TileContext(nc) as tc:
        with (
            tc.tile_pool(name="singles", bufs=1) as singles,
            tc.tile_pool(name="xpool", bufs=1) as xpool,
            tc.tile_pool(name="hsp", bufs=1) as hsp,
            tc.tile_pool(name="qk", bufs=1) as qk,
            tc.tile_pool(name="vpool", bufs=1) as vpool,
            tc.tile_pool(name="espool", bufs=2) as espool,
            tc.tile_pool(name="small", bufs=2) as small,
            tc.tile_pool(name="zf", bufs=2) as zf,
            tc.tile_pool(name="ps", bufs=2, space="PSUM") as ps,    # 2x2 banks
            tc.tile_pool(name="po", bufs=2, space="PSUM") as po,    # 2x1 bank
            tc.tile_pool(name="pz", bufs=1, space="PSUM") as pz,    # 1 bank
            tc.tile_pool(name="pr", bufs=1, space="PSUM") as pr,    # 1 bank
        ):
            # ---- x load first (chunked; stats pipeline behind chunks) ----
            xtb = xpool.tile([P, 2, N], BF16, tag="xtb")
            _dmae = [nc.sync, nc.scalar]
            for t in range(2):
                for h in range(2):
                    _dmae[h].dma_start(
                        xtb[:, t, h * 2048:(h + 1) * 2048],
                        xtb_d[t, :, h * 2048:(h + 1) * 2048])
            # residual input; needed late (strip tails) so queued after xtb
            xr = xpool.tile([P, 2, N], F32, tag="xr")
            for t in range(2):
                for h in range(2):
                    _dmae[h].dma_start(
                        xr[:, t, h * 2048:(h + 1) * 2048],
                        xr_d[t, :, h * 2048:(h + 1) * 2048])

            # ---- weights / constants (gpsimd software queues) ----
            wk = singles.tile([P, 2, C], BF16)
            for ko in range(2):
                nc.gpsimd.dma_start(wk[:, ko, :], wk_d[ko])
            wq = singles.tile([P, 2, C], BF16)
            for ko in range(2):
                nc.gpsimd.dma_start(wq[:, ko, :], wq_d[ko])
            wv = singles.tile([P, 2, C], BF16)
            for ko in range(2):
                nc.gpsimd.dma_start(wv[:, ko, :], wv_d[ko])
            wo = singles.tile([P, 2, C], BF16)
            for ko in range(2):
                nc.gpsimd.dma_start(wo[:, ko, :], wo_d[ko])
            bq = singles.tile([P, 2], F32)
            nc.gpsimd.dma_start(bq[:], bq_d)
            bk = singles.tile([P, 2], F32)
            nc.gpsimd.dma_start(bk[:], bk_d)
            gnw = singles.tile([P, 2], F32)
            nc.gpsimd.dma_start(gnw[:], gnw_d)
            gnb = singles.tile([P, 2], F32)
            nc.gpsimd.dma_start(gnb[:], gnb_d)
            amat = singles.tile([P, P], F32R)
            nc.gpsimd.dma_start(amat[:], amat_d.bitcast(F32R))
            ones1 = singles.tile([1, P], F32R)
            nc.gpsimd.dma_start(ones1[:], ones1_d.bitcast(F32R))
            # bv broadcast to all partitions (stride-0 partition DMA)
            bvrep = singles.tile([P, 4, C], F32)
            bv_b = bass.AP(tensor=bv_d.tensor, offset=bv_d.offset,
                           ap=[[0, P], [1, 4 * C]])
            nc.gpsimd.dma_start(out=bvrep[:].rearrange("p a b -> p (a b)"),
                                in_=bv_b)
            ones8z = singles.tile([P, 2, 16], FP8)
            nc.vector.memset(ones8z[:], 1.0)
            noff = singles.tile([P, 1], F32)
            nc.vector.memset(noff[:], -EOFF)
            epsap = singles.tile([P, 1], F32)
            nc.vector.memset(epsap[:], EPS)

            # ---- GroupNorm stats (per channel, then 8-chan group aggregate) ----
            mv2 = small.tile([P, 4], F32, tag="gnmv")  # [mu_t0 mu_t1 ex2_t0 ex2_t1]
            for t in range(2):
                st = small.tile([P, 8, 6], F32, tag="gnst", name=f"gnst{t}")
                xre = xtb[:, t, :].rearrange("p (s f) -> p s f", f=512)
                for sg in range(8):
                    nc.vector.bn_stats(st[:, sg, :], xre[:, sg, :])
                mvt = small.tile([P, 2], F32, tag="gnmvt", name=f"gnmvt{t}")
                nc.vector.bn_aggr(mvt[:], st[:])  # [mean, var]
                musq = small.tile([P, 1], F32, tag="gnmusq", name=f"gnmusq{t}")
                nc.vector.tensor_mul(musq[:], mvt[:, 0:1], mvt[:, 0:1])
                nc.vector.tensor_copy(mv2[:, t:t + 1], mvt[:, 0:1])
                nc.vector.tensor_add(mv2[:, 2 + t:3 + t], mvt[:, 1:2], musq[:])
            stats2 = small.tile([P, 4], F32R, tag="gnst2")
            nc.vector.tensor_copy(stats2[:], mv2[:])
            gp = pz.tile([P, 512], F32, tag="pz", name="gnagg")
            nc.tensor.matmul(gp[:, 0:4], amat[:], stats2[:], start=True, stop=True)
            gs = small.tile([P, 4], F32, tag="gnagg2")
            nc.vector.tensor_copy(gs[:], gp[:, 0:4])
            gmusq = small.tile([P, 2], F32, tag="gnmusq2")
            nc.vector.tensor_mul(gmusq[:], gs[:, 0:2], gs[:, 0:2])
            gvar = small.tile([P, 2], F32, tag="gnvar")
            nc.vector.tensor_tensor(gvar[:], gs[:, 2:4], gmusq[:], ALU.subtract)
            # rstd = exp(-0.5 * ln(var + eps)) (same ACT table set as softmax)
            lnv = small.tile([P, 2], F32, tag="gnln")
            nc.scalar.activation(lnv[:], gvar[:], AF.Ln, bias=epsap[:], scale=1.0)
            rstd = small.tile([P, 2], F32, tag="gnrstd")
            nc.scalar.activation(rstd[:], lnv[:], AF.Exp, bias=0.0, scale=-0.5)
            alpha = small.tile([P, 2], F32, tag="gnalpha")
            nc.vector.tensor_mul(alpha[:], rstd[:], gnw[:])
            atmp = small.tile([P, 2], F32, tag="gnatmp")
            nc.vector.tensor_mul(atmp[:], gs[:, 0:2], alpha[:])
            beta = small.tile([P, 2], F32, tag="gnbeta")
            nc.vector.tensor_tensor(beta[:], gnb[:], atmp[:], ALU.subtract)
            hs = hsp.tile([P, 2, N], BF16, tag="hs")
            for t in range(2):
                nc.vector.tensor_scalar(hs[:, t, :], xtb[:, t, :],
                                        alpha[:, t:t + 1], beta[:, t:t + 1],
                                        ALU.mult, ALU.add)

            # ---- projections: k first (strip 0 needs all of k) ----
            kT = qk.tile([P, 2, N], FP8, tag="kT")
            qT = qk.tile([P, 2, NQ], FP8, tag="qT")
            for (wt, bt, dst, ntok) in ((wk, bk, kT, N), (wq, bq, qT, NQ)):
                for ch in range(2):
                    for blk in range(ntok // 512):
                        kp = po.tile([P, 2, NSTRIP], F32, tag="po",
                                     name=f"pj{id(wt)}_{ch}_{blk}")
                        kpf = kp[:].rearrange("p a b -> p (a b)")
                        for ko in range(2):
                            nc.tensor.matmul(
                                kpf, wt[:, ko, ch * P:(ch + 1) * P],
                                hs[:, ko, blk * 512:(blk + 1) * 512],
                                start=(ko == 0), stop=(ko == 1))
                        nc.scalar.activation(
                            dst[:, ch, blk * 512:(blk + 1) * 512], kpf,
                            AF.Identity, bias=bt[:, ch:ch + 1], scale=1.0)
            v = vpool.tile([P, MT, C], FP8)
            for g in range(MT // 4):
                vp = ps.tile([P, 4, NSTRIP], F32, tag="ps", name=f"vp{g}")
                for i in range(4):
                    m = 4 * g + i
                    for ko in range(2):
                        nc.tensor.matmul(vp[:, i, :],
                                         hs[:, ko, m * P:(m + 1) * P],
                                         wv[:, ko, :],
                                         start=(ko == 0), stop=(ko == 1))
                nc.vector.tensor_tensor(v[:, 4 * g:4 * g + 4, :], vp[:],
                                        bvrep[:], ALU.add)

            # ---- attention strips (software-pipelined emission) ----
            es_t = [None] * NS
            zp_t = [None] * NS
            op_t = [None] * NS
            rz_t = [None] * NS
            osb_t = [None] * NS

            def emit_scores_exp(s):
                ns = slice(s * NSTRIP, (s + 1) * NSTRIP)
                es = espool.tile([P, MT, NSTRIP], FP8, tag="es", name=f"es{s}")
                es_t[s] = es
                for j in range(MT // 4):
                    sp = ps.tile([P, 4, NSTRIP], F32, tag="ps", name=f"sp{s}_{j}")
                    for i in range(4):
                        m = 4 * j + i
                        nc.tensor.matmul(sp[:, i, :],
                                         kT[:, :, m * P:(m + 1) * P],
                                         qT[:, :, ns],
                                         start=True, stop=True, perf_mode=DR)
                    nc.scalar.activation(es[:, 4 * j:4 * j + 4, :], sp[:],
                                         AF.Exp, bias=noff[:], scale=ISCALE)

            def emit_zav(s):
                es = es_t[s]
                zp = pz.tile([P, 512], F32, tag="pz", name=f"zp{s}")
                op = po.tile([P, 2, NSTRIP], F32, tag="po", name=f"op{s}")
                zp_t[s], op_t[s] = zp, op
                for j2 in range(MT // 2):
                    e2 = es[:, 2 * j2:2 * j2 + 2, :]
                    nc.tensor.matmul(zp[0:16, 0:NSTRIP], ones8z[:], e2,
                                     start=(j2 == 0), stop=(j2 == MT // 2 - 1),
                                     perf_mode=DR)
                    for ch in range(2):
                        nc.tensor.matmul(op[:, ch, :],
                                         v[:, 2 * j2:2 * j2 + 2,
                                           ch * P:(ch + 1) * P],
                                         e2,
                                         start=(j2 == 0),
                                         stop=(j2 == MT // 2 - 1),
                                         perf_mode=DR)

            def emit_tail_a(s):
                # psum reads that free zp/op for the next strip
                rz = small.tile([1, NSTRIP], F32R, tag="rz", name=f"rz{s}")
                rz_t[s] = rz
                rzf = small.tile([1, NSTRIP], F32, tag="rzf", name=f"rzf{s}")
                with nc.allow_low_precision(reason="~18-bit 1/Z is plenty"):
                    nc.vector.reciprocal_approx_fast(rzf[:], zp_t[s][0:1, 0:NSTRIP])
                    nc.vector.tensor_copy(rz[:], rzf[:])
                osb = small.tile([P, 2, NSTRIP], BF16, tag="osb", name=f"osb{s}")
                osb_t[s] = osb
                nc.vector.tensor_copy(osb[:], op_t[s][:])

            def emit_tail_b(s):
                ns = slice(s * NSTRIP, (s + 1) * NSTRIP)
                rp = pr.tile([P, 512], F32, tag="pr", name=f"rp{s}")
                nc.tensor.matmul(rp[:, 0:NSTRIP], ones1[:],
                                 rz_t[s][:], start=True, stop=True)
                op2 = po.tile([P, 2, NSTRIP], F32, tag="po", name=f"op2_{s}")
                for ch in range(2):
                    for ko in range(2):
                        nc.tensor.matmul(op2[:, ch, :],
                                         wo[:, ko, ch * P:(ch + 1) * P],
                                         osb_t[s][:, ko, :],
                                         start=(ko == 0), stop=(ko == 1))
                rzs = small.tile([P, NSTRIP], F32, tag="rzs", name=f"rzs{s}")
                nc.vector.tensor_copy(rzs[:], rp[:, 0:NSTRIP])
                tt = zf.tile([P, 2, NSTRIP], F32, tag="tt", name=f"tt{s}")
                for ch in range(2):
                    nc.vector.tensor_tensor(tt[:, ch, :], op2[:, ch, :],
                                            rzs[:], ALU.mult)
                fin = zf.tile([P, 2, NSTRIP], F32, tag="fin", name=f"fin{s}")
                nc.vector.tensor_tensor(fin[:], xr[:, :, ns], tt[:], ALU.add)
                for t in range(2):
                    nc.sync.dma_start(out_d[t, :, ns], fin[:, t, :])

            emit_scores_exp(0)
            emit_scores_exp(1)
            for s in range(NS):
                emit_zav(s)
                emit_tail_a(s)
                if s + 2 < NS:
                    emit_scores_exp(s + 2)
                emit_tail_b(s)

    nc.finalize()
    return nc


def _get_nc():
    if "nc" not in _prog_cache:
        _prog_cache["nc"] = _build_nc()
    return _prog_cache["nc"]


def _make_in_maps(x, gn_weight, gn_bias, Wq, bq, Wk, bk, Wv, bv, Wo, bo):
    x = np.asarray(x, dtype=np.float32)
    f32 = lambda a: np.ascontiguousarray(np.asarray(a, dtype=np.float32))
    b16 = lambda a: np.ascontiguousarray(
        np.asarray(a, dtype=np.float32).astype(ml_dtypes.bfloat16))

    def packT(b_vec):  # [256] -> [128, 2] (c_out_in, c_out_half)
        return np.ascontiguousarray(f32(b_vec).reshape(2, P).T)

    amat = np.zeros((P, P), np.float32)
    for g in range(P // GS):
        amat[g * GS:(g + 1) * GS, g * GS:(g + 1) * GS] = 1.0 / GS

    common = {
        "wqT": b16(np.asarray(Wq).T).reshape(2, P, C),
        "wkT": b16(np.asarray(Wk).T).reshape(2, P, C),
        "wvT": b16(np.asarray(Wv).T).reshape(2, P, C),
        "woT": b16(np.asarray(Wo, dtype=np.float32).T * RS2).reshape(2, P, C),
        "bqp": packT(bq),
        "bkp": packT(bk),
        "bv4": np.ascontiguousarray(np.tile(f32(bv).reshape(1, C), (1, 4))),
        "gnw": packT(gn_weight),
        "gnb": packT(gn_bias),
        "amat": amat,
        "ones1": np.ones((1, P), np.float32),
    }

    bo_col = f32(bo).reshape(C, 1)
    in_maps = []
    for core in range(8):
        b, half = core // 2, core % 2
        xt = x[b].reshape(C, N)
        if half:
            xt = np.roll(xt, -NQ, axis=1)
        xrm = ((xt + bo_col) * RS2).astype(np.float32)
        in_maps.append({
            "xtb": np.ascontiguousarray(
                xt.astype(ml_dtypes.bfloat16)).reshape(2, P, N),
            "xr": np.ascontiguousarray(xrm).reshape(2, P, N),
            **common,
        })
    return in_maps


def _assemble(results, B):
    out = np.empty((B, C, N), np.float32)
    for core in range(2 * B):
        b, half = core // 2, core % 2
        out[b, :, half * NQ:(half + 1) * NQ] = results[core]["out"].reshape(C, NQ)
    return out.reshape(B, C, 64, 64)


def kernel(x, gn_weight, gn_bias, Wq, bq, Wk, bk, Wv, bv, Wo, bo):
    x = np.asarray(x, dtype=np.float32)
    in_maps = _make_in_maps(x, gn_weight, gn_bias, Wq, bq, Wk, bk, Wv, bv, Wo, bo)
    nc = _get_nc()
    res = run_bass_kernel_spmd(nc, in_maps, list(range(8)))
    return _assemble(res.results, x.shape[0])
